# revision 17
# baseline (speedup 1.0000x reference)
"""CliffordLinearSimple on 8 Trainium2 NeuronCores.

Math (per reference):
    sv   = x[:, :, SV_IDX]                      # [B, IN_F, 9]  (scalar+vector slots)
    svo  = sv.reshape(B, IN_F*9) @ W.T + b      # [B, OUT_F*9]
    v    = svo.reshape(B, OUT_F, 9)[:, :, 1:]   # [B, OUT_F, 8]
    biv  = v[:, :, IU] * v[:, :, JU]            # [B, OUT_F, 28]
    out[..., SV_IDX] = svo; out[..., BIV_IDX] = biv; rest 0

Distribution: tensor-parallel over OUT_F (row-split W): core c owns out
slots [c*1152, (c+1)*1152).  The device does ONLY the GEMM
C[256, 1152] = svT.T @ W_c in bf16 (fp32 PSUM) and writes C back as
bf16; bias add, the 28 bivector products, and the scatter into the
[256, 1024, 256] multivector output all happen on the host in fp32.

DMA model (measured): a queue dispatches ~1 descriptor / ~36ns
(SWDGE ~44ns), and any DRAM<->SBUF transfer is exactly 128 descriptors
(one per partition), so EVERY transfer costs ~4.6us of queue time no
matter its size.  Throughput therefore scales with per-partition line
size.  The kernel uses FEW, LARGE transfers: svT in 4 chunks, each W
column-tile in 6 k-chunks (up to 14 ktiles = 14KB lines), the narrow
n2 tile in 3 chunks; each queue ends up with ~30-45us of queue time
against a ~70us kernel, so DMA stays off the critical path and the
~360-400 GB/s HBM read ceiling is the only DMA constraint.

Compute is k-outer at per-ktile granularity: all six PSUM accumulators
(2 batch tiles x column tiles 504+504+144) live for the whole kernel;
per ktile the order is n1, n0, n2 to match first-chunk arrival order
(q10's Wn1 + q1's svT0 land first).  PE starts ~13.6us (first-chunk
descriptor latency after the ~9us framework preamble) and must never
gap: a gap resets the PE clock ramp (0.65 -> 1.2 -> 2.4 GHz, ~10us of
continuous execution to reach full speed).  Junk warm-up matmuls
bridge the preamble-to-first-chunk window.

Drain: the last 14 ktiles run m-outer so batch-tile 0's casts + output
DMA (a single [128, 2304B-line] transfer) overlap batch-tile 1's
matmuls; m1's output is split across both HWDGE queues.
"""
import sys

if "/opt/trn_rl_repo" not in sys.path:
    sys.path.insert(0, "/opt/trn_rl_repo")

from contextlib import ExitStack

import ml_dtypes
import numpy as np

import concourse.bass as bass
import concourse.tile as tile
from concourse import bacc, mybir
from concourse.bass_utils import run_bass_kernel_spmd

ALG_DIM = 8
D1 = 9
MV_DIM = 256
B, IN_F, OUT_F = 256, 1024, 1024
POW2 = np.array([2 ** i for i in range(ALG_DIM)])
SV_IDX = np.concatenate([[0], POW2])
IU, JU = np.triu_indices(ALG_DIM, 1)
BIV_IDX = POW2[IU] + POW2[JU]
NCORES = 8
OF = OUT_F // NCORES          # 128 out features per core
N_CORE = OF * D1              # 1152 out slots per core
KT = IN_F * D1 // 128         # 72 k-tiles
BT = 2                        # batch tiles of 128
KTAIL = 58                    # last 14 ktiles run m-outer for the drain

NTILES = (504, 504, 144)
NOFF = [sum(NTILES[:i]) for i in range(len(NTILES))]

# (k0, nk) chunk lists; rings: 0=sync, 1=scalar, 2=gpsimd(SWDGE)
WCHUNKS = [(0, 6), (6, 10), (16, 14), (30, 14), (44, 14), (58, 14)]
N2CHUNKS = [(0, 24), (24, 24), (48, 24)]
SVT_CHUNKS = [(0, 6, 0), (6, 22, 2), (28, 22, 2), (50, 22, 2)]  # (k0, nk, ring)
WARM = 12


def build_core_program():
    f32, bf16 = mybir.dt.float32, mybir.dt.bfloat16

    nc = bacc.Bacc("TRN2", target_bir_lowering=False, debug=False)
    svT_d = nc.dram_tensor("svT", [128, KT, B], bf16, kind="ExternalInput").ap()
    W0_d = nc.dram_tensor("Wr0", [128 * KT * NTILES[0]], bf16, kind="ExternalInput").ap()
    W1_d = nc.dram_tensor("Wr1", [128 * KT * NTILES[1]], bf16, kind="ExternalInput").ap()
    W2_d = nc.dram_tensor("Wr2", [128 * KT * NTILES[2]], bf16, kind="ExternalInput").ap()
    # [p, m*1152 + j] = C[m*128 + p, j]: every partition's output line is
    # contiguous (2304B) -> one 128-descriptor DMA per batch tile
    out_d = nc.dram_tensor("outc", [128, BT * N_CORE], bf16, kind="ExternalOutput").ap()

    with tile.TileContext(nc) as tc:
        with ExitStack() as ctx:
            const = ctx.enter_context(tc.tile_pool(name="const", bufs=1))
            wp0 = ctx.enter_context(tc.tile_pool(name="wp0", bufs=4))
            wp1 = ctx.enter_context(tc.tile_pool(name="wp1", bufs=4))
            wp2 = ctx.enter_context(tc.tile_pool(name="wp2", bufs=3))
            spool = ctx.enter_context(tc.tile_pool(name="spool", bufs=2))
            warmpool = ctx.enter_context(tc.tile_pool(name="warmpool", bufs=1, space="PSUM"))
            pspool = ctx.enter_context(
                tc.tile_pool(name="pspool", bufs=BT * len(NTILES), space="PSUM")
            )
            rings = [nc.sync, nc.scalar, nc.gpsimd]

            svT = const.tile([128, KT, B], bf16)

            # PE warm-up with no DMA deps: junk matmuls bridge the gap from
            # engine start (~8.4us) to first-chunk arrival (~13.6us) so the
            # clock ramp is underway when real work lands
            warm_ps = warmpool.tile([128, 504], f32, name="warm", tag="warm")
            ones = const.tile([1, 128], bf16)
            nc.vector.memset(ones[:], 1.0)
            warm_rhs = const.tile([1, 504], bf16)
            nc.vector.memset(warm_rhs[:], 0.0)
            for _ in range(WARM):
                nc.tensor.matmul(
                    warm_ps[:], ones[:], warm_rhs[:],
                    start=True, stop=True, skip_group_check=True,
                )

            ps = {
                (m, n): pspool.tile([128, NTILES[n]], f32, name=f"ps{m}_{n}", tag="ps")
                for n in range(len(NTILES))
                for m in range(BT)
            }

            # -- DMA issue ------------------------------------------------
            # per-engine program order = queue order.
            # q1(sync):    svT0, Wn0 chunks (k-order), outs
            # q10(scalar): Wn1 chunks (k-order), out half
            # q0(SWDGE):   n2_0, svT1, n2_1, svT2, n2_2, svT3
            def issue_svt(k0, nk, r):
                rings[r].dma_start(svT[:, k0:k0 + nk, :], svT_d[:, k0:k0 + nk, :])

            def issue_w(pool, w_d, nt, k0, nk, r, name):
                cols = nk * nt
                wt = pool.tile([128, cols], bf16, name=name, tag=name)
                off = 128 * k0 * nt
                rings[r].dma_start(
                    wt[:], w_d[off:off + 128 * cols].rearrange("(p c) -> p c", p=128)
                )
                return wt

            issue_svt(*SVT_CHUNKS[0])
            w0_tiles = []
            w1_tiles = []
            w2_tiles = []
            # interleave issue so early chunks of every stream are queued
            # before late ones; per-queue order is what matters
            w1_tiles.append((WCHUNKS[0], issue_w(wp1, W1_d, NTILES[1], *WCHUNKS[0], 1, "w1")))
            w2_tiles.append((N2CHUNKS[0], issue_w(wp2, W2_d, NTILES[2], *N2CHUNKS[0], 2, "w2")))
            w0_tiles.append((WCHUNKS[0], issue_w(wp0, W0_d, NTILES[0], *WCHUNKS[0], 0, "w0")))
            issue_svt(*SVT_CHUNKS[1])
            for i in range(1, len(WCHUNKS)):
                w1_tiles.append((WCHUNKS[i], issue_w(wp1, W1_d, NTILES[1], *WCHUNKS[i], 1, "w1")))
                w0_tiles.append((WCHUNKS[i], issue_w(wp0, W0_d, NTILES[0], *WCHUNKS[i], 0, "w0")))
                if i < len(N2CHUNKS):
                    w2_tiles.append((N2CHUNKS[i], issue_w(wp2, W2_d, NTILES[2], *N2CHUNKS[i], 2, "w2")))
                if i + 1 < len(SVT_CHUNKS):
                    issue_svt(*SVT_CHUNKS[i + 1])

            chunk_maps = {0: w0_tiles, 1: w1_tiles, 2: w2_tiles}

            def wslice(n, kt):
                for (k0, nk), wt in chunk_maps[n]:
                    if k0 <= kt < k0 + nk:
                        nt = NTILES[n]
                        return wt[:, (kt - k0) * nt:(kt - k0 + 1) * nt]
                raise AssertionError(f"no chunk for n={n} kt={kt}")

            # -- compute: k-outer, per-ktile; order n1,n0,n2 matches the
            # arrival order of the first chunks -------------------------------
            def mm(kt, n, m):
                nc.tensor.matmul(
                    ps[(m, n)][:],
                    svT[:, kt, m * 128:(m + 1) * 128],
                    wslice(n, kt),
                    start=(kt == 0),
                    stop=(kt == KT - 1),
                )

            for kt in range(KTAIL):
                for n in (1, 0, 2):
                    for m in range(BT):
                        mm(kt, n, m)

            # -- drain: m-outer over the tail so m=0's casts + output DMA
            # overlap m=1's matmuls --------------------------------------
            for m in range(BT):
                for n in (1, 0, 2):
                    for kt in range(KTAIL, KT):
                        mm(kt, n, m)
                st = spool.tile([128, N_CORE], bf16, name=f"st{m}", tag="st")
                for n in range(len(NTILES)):
                    nc.vector.tensor_copy(
                        st[:, NOFF[n]:NOFF[n] + NTILES[n]], ps[(m, n)][:]
                    )
                if m == 0:
                    rings[0].dma_start(out_d[:, :N_CORE], st[:])
                else:
                    h = N_CORE // 2
                    rings[0].dma_start(out_d[:, N_CORE:N_CORE + h], st[:, :h])
                    rings[1].dma_start(out_d[:, N_CORE + h:], st[:, h:])

    nc.finalize()
    return nc


_PROGRAM = None


def _get_program():
    global _PROGRAM
    if _PROGRAM is None:
        _PROGRAM = build_core_program()
    return _PROGRAM


def _prep_inputs(x, W, b):
    bf16 = ml_dtypes.bfloat16
    # svT[p, kt, m] = sv[m, kt*128 + p], sv = x[:, :, SV_IDX] flattened
    sv = np.ascontiguousarray(x[:, :, SV_IDX]).reshape(B, IN_F * D1)
    svT = np.ascontiguousarray(sv.reshape(B, KT, 128).transpose(2, 1, 0)).astype(bf16)

    Wb = W.astype(bf16).reshape(NCORES, N_CORE, KT, 128)
    chunk_lists = {0: WCHUNKS, 1: WCHUNKS, 2: N2CHUNKS}
    in_maps = []
    for c in range(NCORES):
        m = {"svT": svT}
        for n in range(3):
            # chunks packed contiguously in k order, each chunk laid out
            # [p][kl][col] to match the device-side [128, nk*nt] tile
            blk = Wb[c, NOFF[n]:NOFF[n] + NTILES[n]]          # [col, kt, p]
            parts = []
            for k0, nk in chunk_lists[n]:
                sub = blk[:, k0:k0 + nk, :]                   # [col, kl, p]
                parts.append(np.ascontiguousarray(sub.transpose(2, 1, 0)).ravel())
            m[f"Wr{n}"] = np.concatenate(parts)
        in_maps.append(m)
    return in_maps


def run(x, W, b, trace=False):
    x = np.asarray(x, dtype=np.float32)
    W = np.asarray(W, dtype=np.float32)
    b = np.asarray(b, dtype=np.float32)
    in_maps = _prep_inputs(x, W, b)
    nc = _get_program()
    res = None
    for attempt in range(3):
        try:
            res = run_bass_kernel_spmd(
                nc, in_maps, core_ids=list(range(NCORES)), trace=trace
            )
            break
        except Exception:
            if attempt == 2:
                raise
            import time as _time
            _time.sleep(5)
    # host-side epilogue in f32: de-interleave [p, m, j] -> [m*128+p, j],
    # then bias, bivector products, scatter
    svo = np.concatenate(
        [
            np.asarray(res.results[c]["outc"])
            .reshape(128, BT, N_CORE)
            .transpose(1, 0, 2)
            .reshape(B, N_CORE)
            for c in range(NCORES)
        ],
        axis=1,
    ).astype(np.float32)
    svo += b[None, :]
    svo = svo.reshape(B, OUT_F, D1)
    v = svo[:, :, 1:]
    biv = v[:, :, IU] * v[:, :, JU]
    out = np.zeros((B, OUT_F, MV_DIM), dtype=np.float32)
    out[:, :, SV_IDX] = svo
    out[:, :, BIV_IDX] = biv
    return out, res


def kernel(x, W, b):
    out, _ = run(x, W, b)
    return out


# revision 18
# speedup vs baseline: 1.1573x; 1.1573x over previous
"""CliffordLinearSimple on 8 Trainium2 NeuronCores.

Math (per reference):
    sv   = x[:, :, SV_IDX]                      # [B, IN_F, 9]  (scalar+vector slots)
    svo  = sv.reshape(B, IN_F*9) @ W.T + b      # [B, OUT_F*9]
    v    = svo.reshape(B, OUT_F, 9)[:, :, 1:]   # [B, OUT_F, 8]
    biv  = v[:, :, IU] * v[:, :, JU]            # [B, OUT_F, 28]
    out[..., SV_IDX] = svo; out[..., BIV_IDX] = biv; rest 0

Distribution: tensor-parallel over OUT_F (row-split W): core c owns out
slots [c*1152, (c+1)*1152).  The device does ONLY the GEMM
C[256, 1152] = svT.T @ W_c in bf16 (fp32 PSUM) and writes C back as
bf16; bias add, the 28 bivector products, and the scatter into the
[256, 1024, 256] multivector output all happen on the host in fp32.

Measured DMA facts that shape this kernel:
  * any DRAM<->SBUF transfer is 128 descriptors (one per partition);
    a HWDGE queue sustains ~165-190 GB/s with >=10KB lines, much less
    with small lines; SWDGE sustains only ~83 GB/s at any size.
  * aggregate reads cap at ~360-375 GB/s (HBM share), so at full PE
    clock (0.97us/ktile) the stream is a knife-edge against PE demand.
  * PE clock ramps 0.65 -> 1.2 -> 2.4 GHz over ~11us of CONTINUOUS
    work; every stall resets it, so stalls cost triple.

Layout: k-outer at per-ktile granularity over three column tiles of
(468, 432, 252); all six PSUM accumulators (2 batch x 3 columns) live
for the whole kernel.  Queue plan, hand-scheduled so every chunk lands
1-8us before PE's need time assuming 175/175/83 GB/s queue rates:
  q1(sync):    svT(k0-5), n0 k-chunks + svT(k18-29), svT(k44-55), outs
  q10(scalar): n1 k-chunks + svT(k6-17), svT(k30-43), svT(k56-71)
  q0(SWDGE):   the 252-wide n2 stream in 5 chunks (needs 68 GB/s)
Per ktile the compute order is n1, n0, n2 (matches first arrivals).
Junk warm-up matmuls bridge engine-start (~8.4us) to first-chunk
arrival (~13us).  The last 12 ktiles run m-outer so batch-tile 0's
casts + single 128-descriptor output DMA overlap batch-tile 1's
matmuls; m1's output is split across both HWDGE queues.
"""
import sys

if "/opt/trn_rl_repo" not in sys.path:
    sys.path.insert(0, "/opt/trn_rl_repo")

from contextlib import ExitStack

import ml_dtypes
import numpy as np

import concourse.bass as bass
import concourse.tile as tile
from concourse import bacc, mybir
from concourse.bass_utils import run_bass_kernel_spmd

ALG_DIM = 8
D1 = 9
MV_DIM = 256
B, IN_F, OUT_F = 256, 1024, 1024
POW2 = np.array([2 ** i for i in range(ALG_DIM)])
SV_IDX = np.concatenate([[0], POW2])
IU, JU = np.triu_indices(ALG_DIM, 1)
BIV_IDX = POW2[IU] + POW2[JU]
NCORES = 8
OF = OUT_F // NCORES          # 128 out features per core
N_CORE = OF * D1              # 1152 out slots per core
KT = IN_F * D1 // 128         # 72 k-tiles
BT = 2                        # batch tiles of 128
KTAIL = 60                    # last 12 ktiles run m-outer for the drain

NTILES = (468, 432, 252)
NOFF = [sum(NTILES[:i]) for i in range(len(NTILES))]

WCHUNKS = [(0, 6), (6, 8), (14, 10), (24, 12), (36, 12), (48, 12), (60, 12)]
N2CHUNKS = [(0, 8), (8, 12), (20, 16), (36, 18), (54, 18)]
SVT_Q1 = [(0, 6), (18, 12), (44, 12)]
SVT_Q10 = [(6, 12), (30, 14), (56, 16)]
WARM = 12


def build_core_program():
    f32, bf16 = mybir.dt.float32, mybir.dt.bfloat16

    nc = bacc.Bacc("TRN2", target_bir_lowering=False, debug=False)
    svT_d = nc.dram_tensor("svT", [128, KT, B], bf16, kind="ExternalInput").ap()
    W_ds = [
        nc.dram_tensor(f"Wr{n}", [128 * KT * NTILES[n]], bf16, kind="ExternalInput").ap()
        for n in range(3)
    ]
    # [p, m*1152 + j] = C[m*128 + p, j]: every partition's output line is
    # contiguous (2304B) -> one 128-descriptor DMA per batch tile
    out_d = nc.dram_tensor("outc", [128, BT * N_CORE], bf16, kind="ExternalOutput").ap()

    with tile.TileContext(nc) as tc:
        with ExitStack() as ctx:
            const = ctx.enter_context(tc.tile_pool(name="const", bufs=1))
            wp0 = ctx.enter_context(tc.tile_pool(name="wp0", bufs=5))
            wp1 = ctx.enter_context(tc.tile_pool(name="wp1", bufs=5))
            wp2 = ctx.enter_context(tc.tile_pool(name="wp2", bufs=3))
            spool = ctx.enter_context(tc.tile_pool(name="spool", bufs=2))
            warmpool = ctx.enter_context(tc.tile_pool(name="warmpool", bufs=1, space="PSUM"))
            pspool = ctx.enter_context(
                tc.tile_pool(name="pspool", bufs=BT * len(NTILES), space="PSUM")
            )
            rings = [nc.sync, nc.scalar, nc.gpsimd]

            svT = const.tile([128, KT, B], bf16)

            # PE warm-up with no DMA deps: junk matmuls bridge the gap from
            # engine start (~8.4us) to first-chunk arrival so the clock ramp
            # is underway when real work lands
            warm_ps = warmpool.tile([128, 504], f32, name="warm", tag="warm")
            ones = const.tile([1, 128], bf16)
            nc.vector.memset(ones[:], 1.0)
            warm_rhs = const.tile([1, 504], bf16)
            nc.vector.memset(warm_rhs[:], 0.0)
            for _ in range(WARM):
                nc.tensor.matmul(
                    warm_ps[:], ones[:], warm_rhs[:],
                    start=True, stop=True, skip_group_check=True,
                )

            ps = {
                (m, n): pspool.tile([128, NTILES[n]], f32, name=f"ps{m}_{n}", tag="ps")
                for n in range(len(NTILES))
                for m in range(BT)
            }

            # -- DMA issue: per-engine program order = queue order ---------
            pools = {0: wp0, 1: wp1, 2: wp2}

            def issue_svt(k0, nk, r):
                rings[r].dma_start(svT[:, k0:k0 + nk, :], svT_d[:, k0:k0 + nk, :])

            def issue_w(n, ci, r):
                k0, nk = (WCHUNKS if n < 2 else N2CHUNKS)[ci]
                nt = NTILES[n]
                wt = pools[n].tile([128, nk * nt], bf16, name=f"w{n}", tag=f"w{n}")
                off = 128 * k0 * nt
                rings[r].dma_start(
                    wt[:],
                    W_ds[n][off:off + 128 * nk * nt].rearrange("(p c) -> p c", p=128),
                )
                return ((k0, nk), wt)

            w_tiles = {0: [], 1: [], 2: []}
            # interleaved emission; what matters is each engine's own order:
            # q1:  svT(0,6) W0_0 W0_1 W0_2 svT(18,12) W0_3 W0_4 svT(44,12) W0_5 W0_6
            # q10: W1_0 svT(6,12) W1_1 W1_2 W1_3 svT(30,14) W1_4 svT(56,16) W1_5 W1_6
            # q0:  W2_0 W2_1 W2_2 W2_3 W2_4
            issue_svt(*SVT_Q1[0], 0)
            w_tiles[1].append(issue_w(1, 0, 1))
            w_tiles[2].append(issue_w(2, 0, 2))
            w_tiles[0].append(issue_w(0, 0, 0))
            issue_svt(*SVT_Q10[0], 1)
            w_tiles[2].append(issue_w(2, 1, 2))
            w_tiles[0].append(issue_w(0, 1, 0))
            w_tiles[1].append(issue_w(1, 1, 1))
            w_tiles[2].append(issue_w(2, 2, 2))
            w_tiles[0].append(issue_w(0, 2, 0))
            w_tiles[1].append(issue_w(1, 2, 1))
            issue_svt(*SVT_Q1[1], 0)
            w_tiles[1].append(issue_w(1, 3, 1))
            w_tiles[0].append(issue_w(0, 3, 0))
            w_tiles[2].append(issue_w(2, 3, 2))
            issue_svt(*SVT_Q10[1], 1)
            w_tiles[0].append(issue_w(0, 4, 0))
            w_tiles[1].append(issue_w(1, 4, 1))
            w_tiles[2].append(issue_w(2, 4, 2))
            issue_svt(*SVT_Q1[2], 0)
            issue_svt(*SVT_Q10[2], 1)
            w_tiles[0].append(issue_w(0, 5, 0))
            w_tiles[1].append(issue_w(1, 5, 1))
            w_tiles[0].append(issue_w(0, 6, 0))
            w_tiles[1].append(issue_w(1, 6, 1))

            def wslice(n, kt):
                for (k0, nk), wt in w_tiles[n]:
                    if k0 <= kt < k0 + nk:
                        nt = NTILES[n]
                        return wt[:, (kt - k0) * nt:(kt - k0 + 1) * nt]
                raise AssertionError(f"no chunk for n={n} kt={kt}")

            # -- compute: k-outer per ktile, order n1,n0,n2 ----------------
            def mm(kt, n, m):
                nc.tensor.matmul(
                    ps[(m, n)][:],
                    svT[:, kt, m * 128:(m + 1) * 128],
                    wslice(n, kt),
                    start=(kt == 0),
                    stop=(kt == KT - 1),
                )

            for kt in range(KTAIL):
                for n in (1, 0, 2):
                    for m in range(BT):
                        mm(kt, n, m)

            # -- drain: m-outer over the tail so m=0's casts + output DMA
            # overlap m=1's matmuls --------------------------------------
            for m in range(BT):
                for n in (1, 0, 2):
                    for kt in range(KTAIL, KT):
                        mm(kt, n, m)
                st = spool.tile([128, N_CORE], bf16, name=f"st{m}", tag="st")
                for n in range(len(NTILES)):
                    nc.vector.tensor_copy(
                        st[:, NOFF[n]:NOFF[n] + NTILES[n]], ps[(m, n)][:]
                    )
                if m == 0:
                    rings[0].dma_start(out_d[:, :N_CORE], st[:])
                else:
                    h = N_CORE // 2
                    rings[0].dma_start(out_d[:, N_CORE:N_CORE + h], st[:, :h])
                    rings[1].dma_start(out_d[:, N_CORE + h:], st[:, h:])

    nc.finalize()
    return nc


_PROGRAM = None


def _get_program():
    global _PROGRAM
    if _PROGRAM is None:
        _PROGRAM = build_core_program()
    return _PROGRAM


def _prep_inputs(x, W, b):
    bf16 = ml_dtypes.bfloat16
    # svT[p, kt, m] = sv[m, kt*128 + p], sv = x[:, :, SV_IDX] flattened
    sv = np.ascontiguousarray(x[:, :, SV_IDX]).reshape(B, IN_F * D1)
    svT = np.ascontiguousarray(sv.reshape(B, KT, 128).transpose(2, 1, 0)).astype(bf16)

    Wb = W.astype(bf16).reshape(NCORES, N_CORE, KT, 128)
    chunk_lists = {0: WCHUNKS, 1: WCHUNKS, 2: N2CHUNKS}
    in_maps = []
    for c in range(NCORES):
        m = {"svT": svT}
        for n in range(3):
            # chunks packed contiguously in k order, each chunk laid out
            # [p][kl][col] to match the device-side [128, nk*nt] tile
            blk = Wb[c, NOFF[n]:NOFF[n] + NTILES[n]]          # [col, kt, p]
            parts = []
            for k0, nk in chunk_lists[n]:
                sub = blk[:, k0:k0 + nk, :]                   # [col, kl, p]
                parts.append(np.ascontiguousarray(sub.transpose(2, 1, 0)).ravel())
            m[f"Wr{n}"] = np.concatenate(parts)
        in_maps.append(m)
    return in_maps


def run(x, W, b, trace=False):
    x = np.asarray(x, dtype=np.float32)
    W = np.asarray(W, dtype=np.float32)
    b = np.asarray(b, dtype=np.float32)
    in_maps = _prep_inputs(x, W, b)
    nc = _get_program()
    res = None
    for attempt in range(3):
        try:
            res = run_bass_kernel_spmd(
                nc, in_maps, core_ids=list(range(NCORES)), trace=trace
            )
            break
        except Exception:
            if attempt == 2:
                raise
            import time as _time
            _time.sleep(5)
    # host-side epilogue in f32: de-interleave [p, m, j] -> [m*128+p, j],
    # then bias, bivector products, scatter
    svo = np.concatenate(
        [
            np.asarray(res.results[c]["outc"])
            .reshape(128, BT, N_CORE)
            .transpose(1, 0, 2)
            .reshape(B, N_CORE)
            for c in range(NCORES)
        ],
        axis=1,
    ).astype(np.float32)
    svo += b[None, :]
    svo = svo.reshape(B, OUT_F, D1)
    v = svo[:, :, 1:]
    biv = v[:, :, IU] * v[:, :, JU]
    out = np.zeros((B, OUT_F, MV_DIM), dtype=np.float32)
    out[:, :, SV_IDX] = svo
    out[:, :, BIV_IDX] = biv
    return out, res


def kernel(x, W, b):
    out, _ = run(x, W, b)
    return out
